# revision 20
# baseline (speedup 1.0000x reference)
"""Trainium2 Bass kernel: rFFT(65536)->keep 4000 bins->LayerNorm(8000)->Linear(8000,512)->SiLU.

v2: real-input 2-level Cooley-Tukey (no row pairing).  n = 512*n1 + n2,
k = 128*q + k1.  Per row:
  inner:  Y[n2, j] = sum_n1 x[512 n1 + n2] e^{-2 pi i n1 j/128}, j in [0,64];
          F1 cols = [re j=0..64 | im j=1..63] (Yim0 = 0, Y64 real).
  tw:     pa = y*ta -> [P1=Yre*c | 0 | P2=Yim*s], pb = y*tb -> [P3 | 0 | P4]
          per chunk (col 64 zeroed so P2/P4 j=0 slots read 0).
  outer:  X[qt, j] = sum_n2 Z[n2,j] e^{-2 pi i n2 qt/512}, Z = Y*tw, with
          E-weights as lhsT so out partitions = slots (w*64+m), m<32: qt=m,
          m>=32: qt=448+m.  Mirror bins k=128q+(128-j) = conj X[511-q, j].
  b-path: j=64 bins (k=128q+64) via block-level matmul on Y64 (real).
  LN+linear fold into host weights A''[slot, j, o] (sign+mask baked in);
  sum(s) via 65 ap=1 matmuls against w1, sum(s^2) via Act-square then 65
  masked ap=1 matmuls.  Masked/dup slots are zeroed host-side in A''/w1/maskm.
"""

import numpy as np
import ml_dtypes

import concourse.bass as bass
import concourse.tile as tile
from concourse import bacc, mybir
from concourse.bass_utils import run_bass_kernel_spmd

N_CORES = 8
B_FULL = 2048
FFT_N = 65536
KEEP = 4000
EPS = 1e-5

f32 = mybir.dt.float32
bf16 = mybir.dt.bfloat16
ALU = mybir.AluOpType
ACT = mybir.ActivationFunctionType
BF16 = ml_dtypes.bfloat16


# ---------------------------------------------------------------- host consts
def _host_consts():
    n1 = np.arange(128.0)

    F1 = np.zeros((128, 128))
    for t in range(65):
        F1[:, t] = np.cos(2 * np.pi * n1 * t / 128)
    for t in range(65, 128):
        F1[:, t] = -np.sin(2 * np.pi * n1 * (t - 64) / 128)

    ta = np.zeros((128, 512))
    tb = np.zeros((128, 512))
    E = np.zeros((128, 4 * 3 * 128))
    E64 = np.zeros((128, 256))
    qt = np.concatenate([np.arange(32), np.arange(480, 512)]).astype(float)
    for c in range(4):
        n2c = np.arange(c * 128, (c + 1) * 128)[:, None]
        ang = 2 * np.pi * n2c * np.arange(64)[None, :] / FFT_N
        ta[:, c * 128:c * 128 + 64] = np.cos(ang)
        tb[:, c * 128:c * 128 + 64] = -np.sin(ang)
        angh = 2 * np.pi * n2c * np.arange(1, 64)[None, :] / FFT_N
        ta[:, c * 128 + 65:c * 128 + 128] = -np.sin(angh)
        tb[:, c * 128 + 65:c * 128 + 128] = np.cos(angh)

        C = np.cos(2 * np.pi * n2c * qt[None, :] / 512)
        S = -np.sin(2 * np.pi * n2c * qt[None, :] / 512)
        C[:, 32] = 0.0   # qt=480 fully masked (k >= 4000): zero the E col
        S[:, 32] = 0.0
        base = c * 384
        E[:, base + 0:base + 128] = np.concatenate([C, S], axis=1)
        E[:, base + 128:base + 256] = np.concatenate([-C, -S], axis=1)
        E[:, base + 256:base + 384] = np.concatenate([-S, C], axis=1)

        kq = (128 * np.arange(32) + 64)[None, :]
        angb = 2 * np.pi * n2c * kq / FFT_N
        E64[:, c * 64:c * 64 + 32] = np.cos(angb)
        E64[:, c * 64 + 32:c * 64 + 64] = -np.sin(angb)

    return (F1.astype(BF16), ta.astype(BF16), tb.astype(BF16),
            E.astype(BF16), E64.astype(BF16))


def _slot_to_e():
    """(part p, j) -> (e in [0,8000) or -1, sign).  j<=63: p = w*64+m;
    j==64: p = w*32+q for p<64."""
    emap = -np.ones((128, 65), dtype=np.int64)
    smap = np.zeros((128, 65))
    for p in range(128):
        for j in range(65):
            if j == 64:
                if p >= 64:
                    continue
                w, q = divmod(p, 32)
                k = 128 * q + 64
                sign = 1.0
            else:
                w, m = divmod(p, 64)
                if m < 32:
                    k = 128 * m + j
                    sign = 1.0
                else:
                    if j == 0:
                        continue
                    k = 128 * (63 - m + 1) - j
                    sign = -1.0 if w == 1 else 1.0
            if k >= KEEP:
                continue
            emap[p, j] = k + (4000 if w else 0)
            smap[p, j] = sign
    return emap, smap


def _host_linear(ln_w, ln_b, W, b):
    emap, smap = _slot_to_e()
    Af = ln_w[None, :] * W                      # [512, 8000]
    apw = np.zeros((128, 65 * 512))
    w1 = np.zeros((128, 65))
    for j in range(65):
        valid = emap[:, j] >= 0
        e = emap[valid, j]
        apw[valid, j * 512:(j + 1) * 512] = smap[valid, j, None] * Af[:, e].T
        w1[valid, j] = smap[valid, j]
    maskm = (emap >= 0).astype(np.float64)      # [128, 65]
    cvec = Af.sum(axis=1)
    dvec = ln_b @ W.T + b
    cb = np.tile(cvec.astype(np.float32)[None, :], (128, 1))
    db = np.tile(dvec.astype(np.float32)[None, :], (128, 1))
    return apw.astype(BF16), w1.astype(BF16), maskm.astype(BF16), cb, db


# ---------------------------------------------------------------- bass kernel
def build_nc(rows, block, reps=1, sim_safe=False):
    assert rows % block == 0 and block == 128
    nblk = rows // block
    ngrp = rows // 8                 # 8-row DMA groups
    act_out = ACT.Identity if sim_safe else ACT.Silu
    nc = bacc.Bacc("TRN2", target_bir_lowering=False, debug=False)

    xd = nc.dram_tensor("x", [ngrp, 128, 8 * 512], bf16, kind="ExternalInput")
    f1d = nc.dram_tensor("f1", [128, 128], bf16, kind="ExternalInput")
    tad = nc.dram_tensor("ta", [128, 512], bf16, kind="ExternalInput")
    tbd = nc.dram_tensor("tb", [128, 512], bf16, kind="ExternalInput")
    ewd = nc.dram_tensor("ew", [128, 1536], bf16, kind="ExternalInput")
    e64d = nc.dram_tensor("e64", [128, 256], bf16, kind="ExternalInput")
    apwd = nc.dram_tensor("apw", [128, 65 * 512], bf16, kind="ExternalInput")
    w1d = nc.dram_tensor("w1", [128, 65], bf16, kind="ExternalInput")
    mkd = nc.dram_tensor("maskm", [128, 65], bf16, kind="ExternalInput")
    cd = nc.dram_tensor("cvec", [128, 512], f32, kind="ExternalInput")
    dd = nc.dram_tensor("dvec", [128, 512], f32, kind="ExternalInput")
    outd = nc.dram_tensor("out", [nblk, 128, 512], f32, kind="ExternalOutput")

    from contextlib import ExitStack
    import contextlib
    with tile.TileContext(nc) as tc, ExitStack() as es:
        consts = es.enter_context(tc.tile_pool(name="consts", bufs=1))
        f1_sb = consts.tile([128, 128], bf16, name="f1_sb")
        ta_sb = consts.tile([128, 512], bf16, name="ta_sb")
        tb_sb = consts.tile([128, 512], bf16, name="tb_sb")
        ew_sb = consts.tile([128, 1536], bf16, name="ew_sb")
        e64_sb = consts.tile([128, 256], bf16, name="e64_sb")
        mk_sb = consts.tile([128, 65], bf16, name="mk_sb")
        w1_sb = consts.tile([128, 65], bf16, name="w1_sb")
        apw_sb = consts.tile([128, 65 * 512], bf16, name="apw_sb")
        c_sb = consts.tile([128, 512], f32, name="c_sb")
        d_sb = consts.tile([128, 512], f32, name="d_sb")
        for sb, dr in ((f1_sb, f1d), (ta_sb, tad), (tb_sb, tbd),
                       (ew_sb, ewd), (e64_sb, e64d), (mk_sb, mkd),
                       (w1_sb, w1d)):
            nc.sync.dma_start(out=sb, in_=dr[:])
        for sb, dr in ((apw_sb, apwd), (c_sb, cd), (d_sb, dd)):
            nc.gpsimd.dma_start(out=sb, in_=dr[:])

        xp = es.enter_context(tc.tile_pool(name="xp", bufs=3))
        yp = es.enter_context(tc.tile_pool(name="yp", bufs=2, space="PSUM"))
        ysp = es.enter_context(tc.tile_pool(name="ysp", bufs=3))
        pp = es.enter_context(tc.tile_pool(name="pp", bufs=2))
        op = es.enter_context(tc.tile_pool(name="op", bufs=2, space="PSUM"))
        sp = es.enter_context(tc.tile_pool(name="sp", bufs=2))
        sqp = es.enter_context(tc.tile_pool(name="sqp", bufs=1))
        y64p = es.enter_context(tc.tile_pool(name="y64p", bufs=2))
        pm = es.enter_context(tc.tile_pool(name="pm", bufs=1, space="PSUM"))
        pms = es.enter_context(tc.tile_pool(name="pms", bufs=1, space="PSUM"))
        gp = es.enter_context(tc.tile_pool(name="gp", bufs=1, space="PSUM"))
        smp = es.enter_context(tc.tile_pool(name="smp", bufs=2))
        ep = es.enter_context(tc.tile_pool(name="ep", bufs=1))

        loop_ctx = tc.For_i(0, reps, 1) if reps > 1 else contextlib.nullcontext()
        with loop_ctx:
          for blk in range(nblk):
            s_blk = sp.tile([128, 65 * 128], bf16, name="s_blk")
            s3 = s_blk.rearrange("p (j b) -> p j b", j=65)
            y64_blk = y64p.tile([128, 512], bf16, name="y64_blk")
            y64v = y64_blk.rearrange("p (c r) -> p c r", c=4)
            for g in range(16):
                x_t = xp.tile([128, 8 * 512], bf16, name="x_t")
                nc.sync.dma_start(out=x_t, in_=xd[blk * 16 + g])
                o_ps = op.tile([128, 512], f32, name="o_ps")
                for p in range(4):
                    pa = pp.tile([128, 1024], bf16, name="pa")
                    pb = pp.tile([128, 1024], bf16, name="pb")
                    for r2 in range(2):
                        row = 2 * p + r2
                        y_ps = yp.tile([128, 512], f32, name="y_ps")
                        for c in range(4):
                            nc.tensor.matmul(
                                y_ps[:, c * 128:(c + 1) * 128],
                                lhsT=x_t[:, row * 512 + c * 128:
                                         row * 512 + (c + 1) * 128],
                                rhs=f1_sb, start=True, stop=True)
                        y_sb = ysp.tile([128, 512], bf16, name="y_sb")
                        nc.scalar.copy(out=y_sb, in_=y_ps)
                        ysv = y_sb.rearrange("p (c u) -> p c u", c=4)
                        grow = g * 8 + row
                        nc.gpsimd.tensor_copy(
                            out=y64v[:, :, grow:grow + 1],
                            in_=ysv[:, :, 64:65])
                        nc.vector.tensor_mul(
                            pa[:, r2 * 512:(r2 + 1) * 512], y_sb, ta_sb)
                        nc.vector.tensor_mul(
                            pb[:, r2 * 512:(r2 + 1) * 512], y_sb, tb_sb)
                    pav = pa.rearrange("p (r u) -> p r u", r=2)
                    pbv = pb.rearrange("p (r u) -> p r u", r=2)
                    reg = o_ps[:, p * 128:(p + 1) * 128]
                    nmm = 0
                    for c in range(4):
                        for (srcv, w, pl) in ((pav, 0, 0), (pav, 1, 1),
                                              (pbv, 0, 2), (pbv, 1, 2)):
                            nc.tensor.matmul(
                                reg,
                                lhsT=ew_sb[:, c * 384 + pl * 128:
                                           c * 384 + (pl + 1) * 128],
                                rhs=srcv[:, :, c * 128 + w * 64:
                                         c * 128 + (w + 1) * 64],
                                start=(nmm == 0), stop=(nmm == 15))
                            nmm += 1
                ov = o_ps.rearrange("p (pr j) -> p j pr", pr=8)
                nc.vector.tensor_copy(out=s3[:, 0:64, g * 8:(g + 1) * 8],
                                      in_=ov)
            psb = pm.tile([128, 128], f32, name="psb")
            for c in range(4):
                nc.tensor.matmul(psb[0:64, :],
                                 lhsT=e64_sb[:, c * 64:(c + 1) * 64],
                                 rhs=y64_blk[:, c * 128:(c + 1) * 128],
                                 start=(c == 0), stop=(c == 3))
            nc.vector.tensor_copy(out=s3[0:64, 64, :], in_=psb[0:64, :])
            nc.vector.memset(s3[64:128, 64, :], 0.0)
            sq_blk = sqp.tile([128, 65 * 128], bf16, name="sq_blk")
            nc.scalar.activation(sq_blk, s_blk, ACT.Square)
            sq3 = sq_blk.rearrange("p (j b) -> p j b", j=65)
            stat_ps = pms.tile([128, 2], f32, name="stat_ps")
            for j in range(65):
                nc.tensor.matmul(stat_ps[:, 0:1], lhsT=sq3[:, j, :],
                                 rhs=mk_sb[:, j:j + 1],
                                 start=(j == 0), stop=(j == 64))
            for j in range(65):
                nc.tensor.matmul(stat_ps[:, 1:2], lhsT=s3[:, j, :],
                                 rhs=w1_sb[:, j:j + 1],
                                 start=(j == 0), stop=(j == 64))
            g_ps = gp.tile([128, 512], f32, name="g_ps")
            for j in range(65):
                nc.tensor.matmul(g_ps, lhsT=s3[:, j, :],
                                 rhs=apw_sb[:, j * 512:(j + 1) * 512],
                                 start=(j == 0), stop=(j == 64))
            # ---- LN tail
            mu = smp.tile([128, 1], f32, name="mu")
            negmu = smp.tile([128, 1], f32, name="negmu")
            e2 = smp.tile([128, 1], f32, name="e2")
            varep = smp.tile([128, 1], f32, name="varep")
            rec = smp.tile([128, 1], f32, name="rec")
            istd = smp.tile([128, 1], f32, name="istd")
            nc.vector.tensor_scalar_mul(mu, stat_ps[:, 1:2], 1.0 / (2 * KEEP))
            nc.vector.tensor_scalar_mul(negmu, stat_ps[:, 1:2],
                                        -1.0 / (2 * KEEP))
            nc.vector.tensor_scalar_mul(e2, stat_ps[:, 0:1], 1.0 / (2 * KEEP))
            nc.vector.scalar_tensor_tensor(
                out=varep, in0=mu, scalar=negmu, in1=e2,
                op0=ALU.mult, op1=ALU.add)
            nc.vector.tensor_scalar_add(varep, varep, EPS)
            nc.vector.reciprocal(rec, varep)
            nc.scalar.activation(istd, rec, ACT.Sqrt)
            p1 = ep.tile([128, 512], f32, name="p1")
            p2 = ep.tile([128, 512], f32, name="p2")
            o_sb = ep.tile([128, 512], f32, name="o_sb")
            nc.vector.scalar_tensor_tensor(
                out=p1, in0=c_sb, scalar=negmu, in1=g_ps[:, 0:512],
                op0=ALU.mult, op1=ALU.add)
            nc.vector.scalar_tensor_tensor(
                out=p2, in0=p1, scalar=istd, in1=d_sb,
                op0=ALU.mult, op1=ALU.add)
            nc.scalar.activation(o_sb, p2, act_out)
            nc.sync.dma_start(out=outd[blk], in_=o_sb)

    nc.compile()
    return nc


# ---------------------------------------------------------------- entry points
_CACHE = {}


def _get_nc(rows, block, reps=1, sim_safe=False):
    key = (rows, block, reps, sim_safe)
    if key not in _CACHE:
        _CACHE[key] = build_nc(rows, block, reps, sim_safe)
    return _CACHE[key]


def make_in_maps(x, ln_w, ln_b, W, b, rows_per_core, n_cores=N_CORES):
    f1, ta, tb, ew, e64 = _host_consts()
    apw, w1, maskm, cb, db = _host_linear(
        np.asarray(ln_w, np.float64), np.asarray(ln_b, np.float64),
        np.asarray(W, np.float64), np.asarray(b, np.float64))
    xb = np.asarray(x, np.float32).astype(BF16)
    in_maps = []
    for i in range(n_cores):
        xs = xb[i * rows_per_core:(i + 1) * rows_per_core]
        xs = np.ascontiguousarray(
            xs.reshape(rows_per_core // 8, 8, 128, 512)
            .transpose(0, 2, 1, 3).reshape(rows_per_core // 8, 128, 8 * 512))
        in_maps.append({
            "x": xs, "f1": f1, "ta": ta, "tb": tb, "ew": ew, "e64": e64,
            "apw": apw, "w1": w1, "maskm": maskm, "cvec": cb, "dvec": db,
        })
    return in_maps


def run_cores(x, ln_w, ln_b, W, b, rows_per_core, block, n_cores=N_CORES,
              trace=False):
    nc = _get_nc(rows_per_core, block)
    in_maps = make_in_maps(x, ln_w, ln_b, W, b, rows_per_core, n_cores)
    res = run_bass_kernel_spmd(nc, in_maps, core_ids=list(range(n_cores)),
                               trace=trace)
    outs = [res.results[i]["out"].reshape(rows_per_core, 512)
            for i in range(n_cores)]
    return np.concatenate(outs, axis=0), res


def kernel(x, ln_w, ln_b, W, b):
    rows = B_FULL // N_CORES
    out, _ = run_cores(x, ln_w, ln_b, W, b, rows, 128)
    return out.reshape(B_FULL, 1, 512).astype(np.float32)


# revision 22
# speedup vs baseline: 1.0469x; 1.0469x over previous
"""Trainium2 Bass kernel: rFFT(65536)->keep 4000 bins->LayerNorm(8000)->Linear(8000,512)->SiLU.

v2: real-input 2-level Cooley-Tukey (no row pairing).  n = 512*n1 + n2,
k = 128*q + k1.  Per row:
  inner:  Y[n2, j] = sum_n1 x[512 n1 + n2] e^{-2 pi i n1 j/128}, j in [0,64];
          F1 cols = [re j=0..64 | im j=1..63] (Yim0 = 0, Y64 real).
  tw:     pa = y*ta -> [P1=Yre*c | 0 | P2=Yim*s], pb = y*tb -> [P3 | 0 | P4]
          per chunk (col 64 zeroed so P2/P4 j=0 slots read 0).
  outer:  X[qt, j] = sum_n2 Z[n2,j] e^{-2 pi i n2 qt/512}, Z = Y*tw, with
          E-weights as lhsT so out partitions = slots (w*64+m), m<32: qt=m,
          m>=32: qt=448+m.  Mirror bins k=128q+(128-j) = conj X[511-q, j].
  b-path: j=64 bins (k=128q+64) via block-level matmul on Y64 (real).
  LN+linear fold into host weights A''[slot, j, o] (sign+mask baked in);
  sum(s) via 65 ap=1 matmuls against w1, sum(s^2) via Act-square then 65
  masked ap=1 matmuls.  Masked/dup slots are zeroed host-side in A''/w1/maskm.
"""

import numpy as np
import ml_dtypes

import concourse.bass as bass
import concourse.tile as tile
from concourse import bacc, mybir
from concourse.bass_utils import run_bass_kernel_spmd

N_CORES = 8
B_FULL = 2048
FFT_N = 65536
KEEP = 4000
EPS = 1e-5

f32 = mybir.dt.float32
bf16 = mybir.dt.bfloat16
ALU = mybir.AluOpType
ACT = mybir.ActivationFunctionType
BF16 = ml_dtypes.bfloat16


# ---------------------------------------------------------------- host consts
def _host_consts():
    n1 = np.arange(128.0)

    F1 = np.zeros((128, 128))
    for t in range(65):
        F1[:, t] = np.cos(2 * np.pi * n1 * t / 128)
    for t in range(65, 128):
        F1[:, t] = -np.sin(2 * np.pi * n1 * (t - 64) / 128)

    ta = np.zeros((128, 512))
    tb = np.zeros((128, 512))
    E = np.zeros((128, 4 * 3 * 128))
    E64 = np.zeros((128, 256))
    qt = np.concatenate([np.arange(32), np.arange(480, 512)]).astype(float)
    for c in range(4):
        n2c = np.arange(c * 128, (c + 1) * 128)[:, None]
        ang = 2 * np.pi * n2c * np.arange(64)[None, :] / FFT_N
        ta[:, c * 128:c * 128 + 64] = np.cos(ang)
        tb[:, c * 128:c * 128 + 64] = -np.sin(ang)
        angh = 2 * np.pi * n2c * np.arange(1, 64)[None, :] / FFT_N
        ta[:, c * 128 + 65:c * 128 + 128] = -np.sin(angh)
        tb[:, c * 128 + 65:c * 128 + 128] = np.cos(angh)

        C = np.cos(2 * np.pi * n2c * qt[None, :] / 512)
        S = -np.sin(2 * np.pi * n2c * qt[None, :] / 512)
        C[:, 32] = 0.0   # qt=480 fully masked (k >= 4000): zero the E col
        S[:, 32] = 0.0
        base = c * 384
        E[:, base + 0:base + 128] = np.concatenate([C, S], axis=1)
        E[:, base + 128:base + 256] = np.concatenate([-C, -S], axis=1)
        E[:, base + 256:base + 384] = np.concatenate([-S, C], axis=1)

        kq = (128 * np.arange(32) + 64)[None, :]
        angb = 2 * np.pi * n2c * kq / FFT_N
        E64[:, c * 64:c * 64 + 32] = np.cos(angb)
        E64[:, c * 64 + 32:c * 64 + 64] = -np.sin(angb)

    return (F1.astype(BF16), ta.astype(BF16), tb.astype(BF16),
            E.astype(BF16), E64.astype(BF16))


def _slot_to_e():
    """(part p, j) -> (e in [0,8000) or -1, sign).  j<=63: p = w*64+m;
    j==64: p = w*32+q for p<64."""
    emap = -np.ones((128, 65), dtype=np.int64)
    smap = np.zeros((128, 65))
    for p in range(128):
        for j in range(65):
            if j == 64:
                if p >= 64:
                    continue
                w, q = divmod(p, 32)
                k = 128 * q + 64
                sign = 1.0
            else:
                w, m = divmod(p, 64)
                if m < 32:
                    k = 128 * m + j
                    sign = 1.0
                else:
                    if j == 0:
                        continue
                    k = 128 * (63 - m + 1) - j
                    sign = -1.0 if w == 1 else 1.0
            if k >= KEEP:
                continue
            emap[p, j] = k + (4000 if w else 0)
            smap[p, j] = sign
    return emap, smap


def _host_linear(ln_w, ln_b, W, b):
    emap, smap = _slot_to_e()
    Af = ln_w[None, :] * W                      # [512, 8000]
    apw = np.zeros((128, 65 * 512))
    w1 = np.zeros((128, 65))
    for j in range(65):
        valid = emap[:, j] >= 0
        e = emap[valid, j]
        apw[valid, j * 512:(j + 1) * 512] = smap[valid, j, None] * Af[:, e].T
        w1[valid, j] = smap[valid, j]
    maskm = (emap >= 0).astype(np.float64)      # [128, 65]
    cvec = Af.sum(axis=1)
    dvec = ln_b @ W.T + b
    cb = np.tile(cvec.astype(np.float32)[None, :], (128, 1))
    db = np.tile(dvec.astype(np.float32)[None, :], (128, 1))
    return apw.astype(BF16), w1.astype(BF16), maskm.astype(BF16), cb, db


# ---------------------------------------------------------------- bass kernel
def build_nc(rows, block, reps=1, sim_safe=False):
    assert rows % block == 0 and block == 128
    nblk = rows // block
    ngrp = rows // 8                 # 8-row DMA groups
    act_out = ACT.Identity if sim_safe else ACT.Silu
    nc = bacc.Bacc("TRN2", target_bir_lowering=False, debug=False)

    xd = nc.dram_tensor("x", [ngrp, 128, 8 * 512], bf16, kind="ExternalInput")
    f1d = nc.dram_tensor("f1", [128, 128], bf16, kind="ExternalInput")
    tad = nc.dram_tensor("ta", [128, 512], bf16, kind="ExternalInput")
    tbd = nc.dram_tensor("tb", [128, 512], bf16, kind="ExternalInput")
    ewd = nc.dram_tensor("ew", [128, 1536], bf16, kind="ExternalInput")
    e64d = nc.dram_tensor("e64", [128, 256], bf16, kind="ExternalInput")
    apwd = nc.dram_tensor("apw", [128, 65 * 512], bf16, kind="ExternalInput")
    w1d = nc.dram_tensor("w1", [128, 65], bf16, kind="ExternalInput")
    mkd = nc.dram_tensor("maskm", [128, 65], bf16, kind="ExternalInput")
    cd = nc.dram_tensor("cvec", [128, 512], f32, kind="ExternalInput")
    dd = nc.dram_tensor("dvec", [128, 512], f32, kind="ExternalInput")
    outd = nc.dram_tensor("out", [nblk, 128, 512], f32, kind="ExternalOutput")

    from contextlib import ExitStack
    import contextlib
    with tile.TileContext(nc) as tc, ExitStack() as es:
        consts = es.enter_context(tc.tile_pool(name="consts", bufs=1))
        f1_sb = consts.tile([128, 128], bf16, name="f1_sb")
        ta_sb = consts.tile([128, 512], bf16, name="ta_sb")
        tb_sb = consts.tile([128, 512], bf16, name="tb_sb")
        ew_sb = consts.tile([128, 1536], bf16, name="ew_sb")
        e64_sb = consts.tile([128, 256], bf16, name="e64_sb")
        mk_sb = consts.tile([128, 65], bf16, name="mk_sb")
        w1_sb = consts.tile([128, 65], bf16, name="w1_sb")
        apw_sb = consts.tile([128, 65 * 512], bf16, name="apw_sb")
        c_sb = consts.tile([128, 512], f32, name="c_sb")
        d_sb = consts.tile([128, 512], f32, name="d_sb")
        for sb, dr in ((f1_sb, f1d), (ta_sb, tad), (tb_sb, tbd),
                       (ew_sb, ewd), (e64_sb, e64d), (mk_sb, mkd),
                       (w1_sb, w1d)):
            nc.sync.dma_start(out=sb, in_=dr[:])
        for sb, dr in ((apw_sb, apwd), (c_sb, cd), (d_sb, dd)):
            nc.gpsimd.dma_start(out=sb, in_=dr[:])

        xp = es.enter_context(tc.tile_pool(name="xp", bufs=3))
        yp = es.enter_context(tc.tile_pool(name="yp", bufs=3, space="PSUM"))
        ysp = es.enter_context(tc.tile_pool(name="ysp", bufs=3))
        pp = es.enter_context(tc.tile_pool(name="pp", bufs=2))
        op = es.enter_context(tc.tile_pool(name="op", bufs=2, space="PSUM"))
        sp = es.enter_context(tc.tile_pool(name="sp", bufs=2))
        sqp = es.enter_context(tc.tile_pool(name="sqp", bufs=1))
        y64p = es.enter_context(tc.tile_pool(name="y64p", bufs=2))
        pm = es.enter_context(tc.tile_pool(name="pm", bufs=1, space="PSUM"))
        pms = es.enter_context(tc.tile_pool(name="pms", bufs=1, space="PSUM"))
        gp = es.enter_context(tc.tile_pool(name="gp", bufs=1, space="PSUM"))
        smp = es.enter_context(tc.tile_pool(name="smp", bufs=2))
        ep = es.enter_context(tc.tile_pool(name="ep", bufs=1))

        loop_ctx = tc.For_i(0, reps, 1) if reps > 1 else contextlib.nullcontext()
        with loop_ctx:
          for blk in range(nblk):
            s_blk = sp.tile([128, 65 * 128], bf16, name="s_blk")
            s3 = s_blk.rearrange("p (j b) -> p j b", j=65)
            y64_blk = y64p.tile([128, 512], bf16, name="y64_blk")
            y64v = y64_blk.rearrange("p (c r) -> p c r", c=4)
            for g in range(16):
                x_t = xp.tile([128, 8 * 512], bf16, name="x_t")
                nc.sync.dma_start(out=x_t, in_=xd[blk * 16 + g])
                o_ps = op.tile([128, 512], f32, name="o_ps")
                for hf in range(4):
                    pa = pp.tile([128, 1024], bf16, name="pa")
                    pb = pp.tile([128, 1024], bf16, name="pb")
                    for r4 in range(2):
                        row = 2 * hf + r4
                        y_ps = yp.tile([128, 512], f32, name="y_ps")
                        for c in range(4):
                            nc.tensor.matmul(
                                y_ps[:, c * 128:(c + 1) * 128],
                                lhsT=x_t[:, row * 512 + c * 128:
                                         row * 512 + (c + 1) * 128],
                                rhs=f1_sb, start=True, stop=True)
                        y_sb = ysp.tile([128, 512], bf16, name="y_sb")
                        nc.scalar.copy(out=y_sb, in_=y_ps)
                        ysv = y_sb.rearrange("p (c u) -> p c u", c=4)
                        grow = g * 8 + row
                        nc.gpsimd.tensor_copy(
                            out=y64v[:, :, grow:grow + 1],
                            in_=ysv[:, :, 64:65])
                        nc.vector.tensor_mul(
                            pa[:, r4 * 512:(r4 + 1) * 512], y_sb, ta_sb)
                        nc.vector.tensor_mul(
                            pb[:, r4 * 512:(r4 + 1) * 512], y_sb, tb_sb)
                    pav = pa.rearrange("p (r u) -> p r u", r=2)
                    pbv = pb.rearrange("p (r u) -> p r u", r=2)
                    reg = o_ps[:, hf * 128:(hf + 1) * 128]
                    nmm = 0
                    for c in range(4):
                        for (srcv, w, pl) in ((pav, 0, 0), (pav, 1, 1),
                                              (pbv, 0, 2), (pbv, 1, 2)):
                            nc.tensor.matmul(
                                reg,
                                lhsT=ew_sb[:, c * 384 + pl * 128:
                                           c * 384 + (pl + 1) * 128],
                                rhs=srcv[:, :, c * 128 + w * 64:
                                         c * 128 + (w + 1) * 64],
                                start=(nmm == 0), stop=(nmm == 15))
                            nmm += 1
                ov = o_ps.rearrange("p (pr j) -> p j pr", pr=8)
                nc.vector.tensor_copy(out=s3[:, 0:64, g * 8:(g + 1) * 8],
                                      in_=ov)
            psb = pm.tile([128, 128], f32, name="psb")
            for c in range(4):
                nc.tensor.matmul(psb[0:64, :],
                                 lhsT=e64_sb[:, c * 64:(c + 1) * 64],
                                 rhs=y64_blk[:, c * 128:(c + 1) * 128],
                                 start=(c == 0), stop=(c == 3))
            nc.vector.tensor_copy(out=s3[0:64, 64, :], in_=psb[0:64, :])
            nc.vector.memset(s3[64:128, 64, :], 0.0)
            sq_blk = sqp.tile([128, 65 * 128], bf16, name="sq_blk")
            nc.scalar.activation(sq_blk, s_blk, ACT.Square)
            sq3 = sq_blk.rearrange("p (j b) -> p j b", j=65)
            stat_ps = pms.tile([128, 2], f32, name="stat_ps")
            for j in range(65):
                nc.tensor.matmul(stat_ps[:, 0:1], lhsT=sq3[:, j, :],
                                 rhs=mk_sb[:, j:j + 1],
                                 start=(j == 0), stop=(j == 64))
            for j in range(65):
                nc.tensor.matmul(stat_ps[:, 1:2], lhsT=s3[:, j, :],
                                 rhs=w1_sb[:, j:j + 1],
                                 start=(j == 0), stop=(j == 64))
            g_ps = gp.tile([128, 512], f32, name="g_ps")
            for j in range(65):
                nc.tensor.matmul(g_ps, lhsT=s3[:, j, :],
                                 rhs=apw_sb[:, j * 512:(j + 1) * 512],
                                 start=(j == 0), stop=(j == 64))
            # ---- LN tail
            mu = smp.tile([128, 1], f32, name="mu")
            negmu = smp.tile([128, 1], f32, name="negmu")
            e2 = smp.tile([128, 1], f32, name="e2")
            varep = smp.tile([128, 1], f32, name="varep")
            rec = smp.tile([128, 1], f32, name="rec")
            istd = smp.tile([128, 1], f32, name="istd")
            nc.vector.tensor_scalar_mul(mu, stat_ps[:, 1:2], 1.0 / (2 * KEEP))
            nc.vector.tensor_scalar_mul(negmu, stat_ps[:, 1:2],
                                        -1.0 / (2 * KEEP))
            nc.vector.tensor_scalar_mul(e2, stat_ps[:, 0:1], 1.0 / (2 * KEEP))
            nc.vector.scalar_tensor_tensor(
                out=varep, in0=mu, scalar=negmu, in1=e2,
                op0=ALU.mult, op1=ALU.add)
            nc.vector.tensor_scalar_add(varep, varep, EPS)
            nc.vector.reciprocal(rec, varep)
            nc.scalar.activation(istd, rec, ACT.Sqrt)
            p1 = ep.tile([128, 512], f32, name="p1")
            p2 = ep.tile([128, 512], f32, name="p2")
            o_sb = ep.tile([128, 512], f32, name="o_sb")
            nc.vector.scalar_tensor_tensor(
                out=p1, in0=c_sb, scalar=negmu, in1=g_ps[:, 0:512],
                op0=ALU.mult, op1=ALU.add)
            nc.vector.scalar_tensor_tensor(
                out=p2, in0=p1, scalar=istd, in1=d_sb,
                op0=ALU.mult, op1=ALU.add)
            nc.scalar.activation(o_sb, p2, act_out)
            nc.sync.dma_start(out=outd[blk], in_=o_sb)

    nc.compile()
    return nc


# ---------------------------------------------------------------- entry points
_CACHE = {}


def _get_nc(rows, block, reps=1, sim_safe=False):
    key = (rows, block, reps, sim_safe)
    if key not in _CACHE:
        _CACHE[key] = build_nc(rows, block, reps, sim_safe)
    return _CACHE[key]


def make_in_maps(x, ln_w, ln_b, W, b, rows_per_core, n_cores=N_CORES):
    f1, ta, tb, ew, e64 = _host_consts()
    apw, w1, maskm, cb, db = _host_linear(
        np.asarray(ln_w, np.float64), np.asarray(ln_b, np.float64),
        np.asarray(W, np.float64), np.asarray(b, np.float64))
    xb = np.asarray(x, np.float32).astype(BF16)
    in_maps = []
    for i in range(n_cores):
        xs = xb[i * rows_per_core:(i + 1) * rows_per_core]
        xs = np.ascontiguousarray(
            xs.reshape(rows_per_core // 8, 8, 128, 512)
            .transpose(0, 2, 1, 3).reshape(rows_per_core // 8, 128, 8 * 512))
        in_maps.append({
            "x": xs, "f1": f1, "ta": ta, "tb": tb, "ew": ew, "e64": e64,
            "apw": apw, "w1": w1, "maskm": maskm, "cvec": cb, "dvec": db,
        })
    return in_maps


def run_cores(x, ln_w, ln_b, W, b, rows_per_core, block, n_cores=N_CORES,
              trace=False):
    nc = _get_nc(rows_per_core, block)
    in_maps = make_in_maps(x, ln_w, ln_b, W, b, rows_per_core, n_cores)
    res = run_bass_kernel_spmd(nc, in_maps, core_ids=list(range(n_cores)),
                               trace=trace)
    outs = [res.results[i]["out"].reshape(rows_per_core, 512)
            for i in range(n_cores)]
    return np.concatenate(outs, axis=0), res


def kernel(x, ln_w, ln_b, W, b):
    rows = B_FULL // N_CORES
    out, _ = run_cores(x, ln_w, ln_b, W, b, rows, 128)
    return out.reshape(B_FULL, 1, 512).astype(np.float32)


# revision 26
# speedup vs baseline: 1.0472x; 1.0002x over previous
"""Trainium2 Bass kernel: rFFT(65536)->keep 4000 bins->LayerNorm(8000)->Linear(8000,512)->SiLU.

v2: real-input 2-level Cooley-Tukey (no row pairing).  n = 512*n1 + n2,
k = 128*q + k1.  Per row:
  inner:  Y[n2, j] = sum_n1 x[512 n1 + n2] e^{-2 pi i n1 j/128}, j in [0,64];
          F1 cols = [re j=0..64 | im j=1..63] (Yim0 = 0, Y64 real).
  tw:     pa = y*ta -> [P1=Yre*c | 0 | P2=Yim*s], pb = y*tb -> [P3 | 0 | P4]
          per chunk (col 64 zeroed so P2/P4 j=0 slots read 0).
  outer:  X[qt, j] = sum_n2 Z[n2,j] e^{-2 pi i n2 qt/512}, Z = Y*tw, with
          E-weights as lhsT so out partitions = slots (w*64+m), m<32: qt=m,
          m>=32: qt=448+m.  Mirror bins k=128q+(128-j) = conj X[511-q, j].
  b-path: j=64 bins (k=128q+64) via block-level matmul on Y64 (real).
  LN+linear fold into host weights A''[slot, j, o] (sign+mask baked in);
  sum(s) via 65 ap=1 matmuls against w1, sum(s^2) via Act-square then 65
  masked ap=1 matmuls.  Masked/dup slots are zeroed host-side in A''/w1/maskm.
"""

import numpy as np
import ml_dtypes

import concourse.bass as bass
import concourse.tile as tile
from concourse import bacc, mybir
from concourse.bass_utils import run_bass_kernel_spmd

N_CORES = 8
B_FULL = 2048
FFT_N = 65536
KEEP = 4000
EPS = 1e-5

f32 = mybir.dt.float32
bf16 = mybir.dt.bfloat16
ALU = mybir.AluOpType
ACT = mybir.ActivationFunctionType
BF16 = ml_dtypes.bfloat16


# ---------------------------------------------------------------- host consts
def _host_consts():
    n1 = np.arange(128.0)

    F1 = np.zeros((128, 128))
    for t in range(65):
        F1[:, t] = np.cos(2 * np.pi * n1 * t / 128)
    for t in range(65, 128):
        F1[:, t] = -np.sin(2 * np.pi * n1 * (t - 64) / 128)

    ta = np.zeros((128, 512))
    tb = np.zeros((128, 512))
    E = np.zeros((128, 4 * 3 * 128))
    E64 = np.zeros((128, 256))
    qt = np.concatenate([np.arange(32), np.arange(480, 512)]).astype(float)
    for c in range(4):
        n2c = np.arange(c * 128, (c + 1) * 128)[:, None]
        ang = 2 * np.pi * n2c * np.arange(64)[None, :] / FFT_N
        ta[:, c * 128:c * 128 + 64] = np.cos(ang)
        tb[:, c * 128:c * 128 + 64] = -np.sin(ang)
        angh = 2 * np.pi * n2c * np.arange(1, 64)[None, :] / FFT_N
        ta[:, c * 128 + 65:c * 128 + 128] = -np.sin(angh)
        tb[:, c * 128 + 65:c * 128 + 128] = np.cos(angh)

        C = np.cos(2 * np.pi * n2c * qt[None, :] / 512)
        S = -np.sin(2 * np.pi * n2c * qt[None, :] / 512)
        C[:, 32] = 0.0   # qt=480 fully masked (k >= 4000): zero the E col
        S[:, 32] = 0.0
        base = c * 384
        E[:, base + 0:base + 128] = np.concatenate([C, S], axis=1)
        E[:, base + 128:base + 256] = np.concatenate([-C, -S], axis=1)
        E[:, base + 256:base + 384] = np.concatenate([-S, C], axis=1)

        kq = (128 * np.arange(32) + 64)[None, :]
        angb = 2 * np.pi * n2c * kq / FFT_N
        E64[:, c * 64:c * 64 + 32] = np.cos(angb)
        E64[:, c * 64 + 32:c * 64 + 64] = -np.sin(angb)

    return (F1.astype(BF16), ta.astype(BF16), tb.astype(BF16),
            E.astype(BF16), E64.astype(BF16))


def _slot_to_e():
    """(part p, j) -> (e in [0,8000) or -1, sign).  j<=63: p = w*64+m;
    j==64: p = w*32+q for p<64."""
    emap = -np.ones((128, 65), dtype=np.int64)
    smap = np.zeros((128, 65))
    for p in range(128):
        for j in range(65):
            if j == 64:
                if p >= 64:
                    continue
                w, q = divmod(p, 32)
                k = 128 * q + 64
                sign = 1.0
            else:
                w, m = divmod(p, 64)
                if m < 32:
                    k = 128 * m + j
                    sign = 1.0
                else:
                    if j == 0:
                        continue
                    k = 128 * (63 - m + 1) - j
                    sign = -1.0 if w == 1 else 1.0
            if k >= KEEP:
                continue
            emap[p, j] = k + (4000 if w else 0)
            smap[p, j] = sign
    return emap, smap


def _host_linear(ln_w, ln_b, W, b):
    emap, smap = _slot_to_e()
    Af = ln_w[None, :] * W                      # [512, 8000]
    apw = np.zeros((128, 65 * 512))
    w1 = np.zeros((128, 65))
    for j in range(65):
        valid = emap[:, j] >= 0
        e = emap[valid, j]
        apw[valid, j * 512:(j + 1) * 512] = smap[valid, j, None] * Af[:, e].T
        w1[valid, j] = smap[valid, j]
    maskm = (emap >= 0).astype(np.float64)      # [128, 65]
    cvec = Af.sum(axis=1)
    dvec = ln_b @ W.T + b
    cb = np.tile(cvec.astype(np.float32)[None, :], (128, 1))
    db = np.tile(dvec.astype(np.float32)[None, :], (128, 1))
    return apw.astype(BF16), w1.astype(BF16), maskm.astype(BF16), cb, db


# ---------------------------------------------------------------- bass kernel
def build_nc(rows, block, reps=1, sim_safe=False):
    assert rows % block == 0 and block == 128
    nblk = rows // block
    ngrp = rows // 8                 # 8-row DMA groups
    act_out = ACT.Identity if sim_safe else ACT.Silu
    nc = bacc.Bacc("TRN2", target_bir_lowering=False, debug=False)

    xd = nc.dram_tensor("x", [ngrp, 128, 8 * 512], bf16, kind="ExternalInput")
    f1d = nc.dram_tensor("f1", [128, 128], bf16, kind="ExternalInput")
    tad = nc.dram_tensor("ta", [128, 512], bf16, kind="ExternalInput")
    tbd = nc.dram_tensor("tb", [128, 512], bf16, kind="ExternalInput")
    ewd = nc.dram_tensor("ew", [128, 1536], bf16, kind="ExternalInput")
    e64d = nc.dram_tensor("e64", [128, 256], bf16, kind="ExternalInput")
    apwd = nc.dram_tensor("apw", [128, 65 * 512], bf16, kind="ExternalInput")
    w1d = nc.dram_tensor("w1", [128, 65], bf16, kind="ExternalInput")
    mkd = nc.dram_tensor("maskm", [128, 65], bf16, kind="ExternalInput")
    cd = nc.dram_tensor("cvec", [128, 512], f32, kind="ExternalInput")
    dd = nc.dram_tensor("dvec", [128, 512], f32, kind="ExternalInput")
    outd = nc.dram_tensor("out", [nblk, 128, 512], f32, kind="ExternalOutput")

    from contextlib import ExitStack
    import contextlib
    with tile.TileContext(nc) as tc, ExitStack() as es:
        consts = es.enter_context(tc.tile_pool(name="consts", bufs=1))
        f1_sb = consts.tile([128, 128], bf16, name="f1_sb")
        ta_sb = consts.tile([128, 512], bf16, name="ta_sb")
        tb_sb = consts.tile([128, 512], bf16, name="tb_sb")
        ew_sb = consts.tile([128, 1536], bf16, name="ew_sb")
        e64_sb = consts.tile([128, 256], bf16, name="e64_sb")
        mk_sb = consts.tile([128, 65], bf16, name="mk_sb")
        w1_sb = consts.tile([128, 65], bf16, name="w1_sb")
        apw_sb = consts.tile([128, 65 * 512], bf16, name="apw_sb")
        c_sb = consts.tile([128, 512], f32, name="c_sb")
        d_sb = consts.tile([128, 512], f32, name="d_sb")
        for sb, dr in ((f1_sb, f1d), (ta_sb, tad), (tb_sb, tbd),
                       (ew_sb, ewd), (e64_sb, e64d), (mk_sb, mkd),
                       (w1_sb, w1d)):
            nc.sync.dma_start(out=sb, in_=dr[:])
        for sb, dr in ((apw_sb, apwd), (c_sb, cd), (d_sb, dd)):
            nc.gpsimd.dma_start(out=sb, in_=dr[:])

        xp = es.enter_context(tc.tile_pool(name="xp", bufs=3))
        yp = es.enter_context(tc.tile_pool(name="yp", bufs=3, space="PSUM"))
        ysp = es.enter_context(tc.tile_pool(name="ysp", bufs=3))
        pp = es.enter_context(tc.tile_pool(name="pp", bufs=2))
        op = es.enter_context(tc.tile_pool(name="op", bufs=2, space="PSUM"))
        sp = es.enter_context(tc.tile_pool(name="sp", bufs=2))
        sqp = es.enter_context(tc.tile_pool(name="sqp", bufs=1))
        y64p = es.enter_context(tc.tile_pool(name="y64p", bufs=2))
        pm = es.enter_context(tc.tile_pool(name="pm", bufs=1, space="PSUM"))
        pms = es.enter_context(tc.tile_pool(name="pms", bufs=1, space="PSUM"))
        gp = es.enter_context(tc.tile_pool(name="gp", bufs=1, space="PSUM"))
        smp = es.enter_context(tc.tile_pool(name="smp", bufs=2))
        ep = es.enter_context(tc.tile_pool(name="ep", bufs=1))

        loop_ctx = tc.For_i(0, reps, 1) if reps > 1 else contextlib.nullcontext()
        with loop_ctx:
          for blk in range(nblk):
            s_blk = sp.tile([128, 65 * 128], bf16, name="s_blk")
            s3 = s_blk.rearrange("p (j b) -> p j b", j=65)
            y64_blk = y64p.tile([128, 512], bf16, name="y64_blk")
            y64v = y64_blk.rearrange("p (c r) -> p c r", c=4)
            for g in range(16):
                x_t = xp.tile([128, 8 * 512], bf16, name="x_t")
                nc.sync.dma_start(out=x_t, in_=xd[blk * 16 + g])
                o_ps = op.tile([128, 512], f32, name="o_ps")
                for hf in range(4):
                    pa = pp.tile([128, 1024], bf16, name="pa")
                    pb = pp.tile([128, 1024], bf16, name="pb")
                    for r4 in range(2):
                        row = 2 * hf + r4
                        y_ps = yp.tile([128, 512], f32, name="y_ps")
                        for c in range(4):
                            nc.tensor.matmul(
                                y_ps[:, c * 128:(c + 1) * 128],
                                lhsT=x_t[:, row * 512 + c * 128:
                                         row * 512 + (c + 1) * 128],
                                rhs=f1_sb, start=True, stop=True)
                        y_sb = ysp.tile([128, 512], bf16, name="y_sb")
                        nc.scalar.copy(out=y_sb, in_=y_ps)
                        ysv = y_sb.rearrange("p (c u) -> p c u", c=4)
                        grow = g * 8 + row
                        nc.gpsimd.tensor_copy(
                            out=y64v[:, :, grow:grow + 1],
                            in_=ysv[:, :, 64:65])
                        nc.vector.tensor_mul(
                            pa[:, r4 * 512:(r4 + 1) * 512], y_sb, ta_sb)
                        nc.vector.tensor_mul(
                            pb[:, r4 * 512:(r4 + 1) * 512], y_sb, tb_sb)
                    pav = pa.rearrange("p (r u) -> p r u", r=2)
                    pbv = pb.rearrange("p (r u) -> p r u", r=2)
                    reg = o_ps[:, hf * 128:(hf + 1) * 128]
                    nmm = 0
                    for c in range(4):
                        for (srcv, w, pl) in ((pav, 0, 0), (pav, 1, 1),
                                              (pbv, 0, 2), (pbv, 1, 2)):
                            nc.tensor.matmul(
                                reg,
                                lhsT=ew_sb[:, c * 384 + pl * 128:
                                           c * 384 + (pl + 1) * 128],
                                rhs=srcv[:, :, c * 128 + w * 64:
                                         c * 128 + (w + 1) * 64],
                                start=(nmm == 0), stop=(nmm == 15))
                            nmm += 1
                ov = o_ps.rearrange("p (pr j) -> p j pr", pr=8)
                nc.vector.tensor_copy(out=s3[:, 0:64, g * 8:(g + 1) * 8],
                                      in_=ov)
            psb = pm.tile([128, 128], f32, name="psb")
            for c in range(4):
                nc.tensor.matmul(psb[0:64, :],
                                 lhsT=e64_sb[:, c * 64:(c + 1) * 64],
                                 rhs=y64_blk[:, c * 128:(c + 1) * 128],
                                 start=(c == 0), stop=(c == 3))
            nc.vector.tensor_copy(out=s3[0:64, 64, :], in_=psb[0:64, :])
            nc.vector.memset(s3[64:128, 64, :], 0.0)
            sq_blk = sqp.tile([128, 65 * 128], bf16, name="sq_blk")
            nc.scalar.activation(sq_blk, s_blk, ACT.Square)
            sq3 = sq_blk.rearrange("p (j b) -> p j b", j=65)
            stat_ps = pms.tile([128, 2], f32, name="stat_ps")
            for j in range(65):
                nc.tensor.matmul(stat_ps[:, 0:1], lhsT=sq3[:, j, :],
                                 rhs=mk_sb[:, j:j + 1],
                                 start=(j == 0), stop=(j == 64))
            for j in range(65):
                nc.tensor.matmul(stat_ps[:, 1:2], lhsT=s3[:, j, :],
                                 rhs=w1_sb[:, j:j + 1],
                                 start=(j == 0), stop=(j == 64))
            g_ps = gp.tile([128, 512], f32, name="g_ps")
            for j in range(65):
                nc.tensor.matmul(g_ps, lhsT=s3[:, j, :],
                                 rhs=apw_sb[:, j * 512:(j + 1) * 512],
                                 start=(j == 0), stop=(j == 64))
            # ---- LN tail
            mu = smp.tile([128, 1], f32, name="mu")
            negmu = smp.tile([128, 1], f32, name="negmu")
            e2 = smp.tile([128, 1], f32, name="e2")
            varep = smp.tile([128, 1], f32, name="varep")
            rec = smp.tile([128, 1], f32, name="rec")
            istd = smp.tile([128, 1], f32, name="istd")
            nc.vector.tensor_scalar_mul(mu, stat_ps[:, 1:2], 1.0 / (2 * KEEP))
            nc.vector.tensor_scalar_mul(negmu, stat_ps[:, 1:2],
                                        -1.0 / (2 * KEEP))
            nc.vector.tensor_scalar_mul(e2, stat_ps[:, 0:1], 1.0 / (2 * KEEP))
            nc.vector.scalar_tensor_tensor(
                out=varep, in0=mu, scalar=negmu, in1=e2,
                op0=ALU.mult, op1=ALU.add)
            nc.vector.tensor_scalar_add(varep, varep, EPS)
            nc.vector.reciprocal(rec, varep)
            nc.scalar.activation(istd, rec, ACT.Sqrt)
            p1 = ep.tile([128, 512], f32, name="p1")
            p2 = ep.tile([128, 512], f32, name="p2")
            o_sb = ep.tile([128, 512], f32, name="o_sb")
            nc.vector.scalar_tensor_tensor(
                out=p1, in0=c_sb, scalar=negmu, in1=g_ps[:, 0:512],
                op0=ALU.mult, op1=ALU.add)
            nc.vector.scalar_tensor_tensor(
                out=p2, in0=p1, scalar=istd, in1=d_sb,
                op0=ALU.mult, op1=ALU.add)
            nc.scalar.activation(o_sb, p2, act_out)
            nc.sync.dma_start(out=outd[blk], in_=o_sb)

    nc.compile()
    return nc


# ---------------------------------------------------------------- entry points
_CACHE = {}


def _get_nc(rows, block, reps=1, sim_safe=False):
    key = (rows, block, reps, sim_safe)
    if key not in _CACHE:
        _CACHE[key] = build_nc(rows, block, reps, sim_safe)
    return _CACHE[key]


def make_in_maps(x, ln_w, ln_b, W, b, rows_per_core, n_cores=N_CORES):
    f1, ta, tb, ew, e64 = _host_consts()
    apw, w1, maskm, cb, db = _host_linear(
        np.asarray(ln_w, np.float64), np.asarray(ln_b, np.float64),
        np.asarray(W, np.float64), np.asarray(b, np.float64))
    xb = np.asarray(x, np.float32).astype(BF16)
    in_maps = []
    for i in range(n_cores):
        xs = xb[i * rows_per_core:(i + 1) * rows_per_core]
        xs = np.ascontiguousarray(
            xs.reshape(rows_per_core // 8, 8, 128, 512)
            .transpose(0, 2, 1, 3).reshape(rows_per_core // 8, 128, 8 * 512))
        in_maps.append({
            "x": xs, "f1": f1, "ta": ta, "tb": tb, "ew": ew, "e64": e64,
            "apw": apw, "w1": w1, "maskm": maskm, "cvec": cb, "dvec": db,
        })
    return in_maps


def run_cores(x, ln_w, ln_b, W, b, rows_per_core, block, n_cores=N_CORES,
              trace=False):
    nc = _get_nc(rows_per_core, block)
    in_maps = make_in_maps(x, ln_w, ln_b, W, b, rows_per_core, n_cores)
    res = run_bass_kernel_spmd(nc, in_maps, core_ids=list(range(n_cores)),
                               trace=trace)
    outs = [res.results[i]["out"].reshape(rows_per_core, 512)
            for i in range(n_cores)]
    return np.concatenate(outs, axis=0), res


def kernel(x, ln_w, ln_b, W, b):
    rows = B_FULL // N_CORES
    out, _ = run_cores(x, ln_w, ln_b, W, b, rows, 128)
    return out.reshape(B_FULL, 1, 512).astype(np.float32)
